# revision 1
# baseline (speedup 1.0000x reference)
"""Trainium2 Bass kernel for nn_DeconvCNNLoss.

Computes  sum_{b,l} exp(s[b,l]/tau) / sum_v exp(dist[b,l,v]/tau)
with  dist = einsum('bel,ve->blv', embed_DE, embed_M)
and   s    = sum_e embed_EN * embed_DE.

Sharding: tensor-parallel over the vocab dim V across 8 cores.  Each core
receives embed_M's shard pre-transposed to [E, V/8] (layout choice made on
the host while sharding), the full embed_DE / embed_EN, and produces
partial exp-sum denominators for all B*L tokens plus the numerator dot
products.  The host sums the 8 partial denominators (the "all-reduce"),
applies exp to the numerator and does the final division + scalar sum.

Device work per core:
  - 512 f32 matmuls [128e,128l]^T @ [128e,500v] accumulated over e in PSUM
  - fused exp+row-sum on the scalar engine (activation Exp with accum_out)
  - numerator: DVE elementwise EN*DE, partition-reduced with a ones-matmul
"""

import numpy as np

B, E, L, V = 4, 512, 512, 32000
NCORES = 8
VS = V // NCORES          # 4000 vocab rows per core
VBLK = 500                # vocab columns per matmul (one PSUM bank)
NVB = VS // VBLK          # 8 vocab blocks per core
NLB = L // 128            # 4 token blocks per batch entry
NTB = B * NLB             # 16 token blocks total
NKB = E // 128            # 4 contraction blocks
INV_TAU = 0.1

_CACHE = {}
LAST_RESULTS = None       # test.py reads exec_time_ns from here


def _build():
    from contextlib import ExitStack

    import concourse.bacc as bacc
    import concourse.mybir as mybir
    import concourse.tile as tile

    f32 = mybir.dt.float32
    nc = bacc.Bacc("TRN2", debug=False, num_devices=NCORES)

    mt = nc.dram_tensor("mt", [E, VS], f32, kind="ExternalInput").ap()
    de = nc.dram_tensor("de", [B, E, L], f32, kind="ExternalInput").ap()
    en = nc.dram_tensor("en", [B, E, L], f32, kind="ExternalInput").ap()
    # down_acc[p, tb*4+h] = sum over one 1000-col vocab slice of exp(dist/tau)
    # for token (b=tb//4, l=(tb%4)*128+p)
    down_acc = nc.dram_tensor("down_acc", [128, NTB * 4], f32, kind="ExternalOutput").ap()
    # s_out[b, l] = sum_e EN[b,e,l]*DE[b,e,l]  (pre-exp numerator dots)
    s_out = nc.dram_tensor("s_out", [B, L], f32, kind="ExternalOutput").ap()

    with tile.TileContext(nc) as tc, ExitStack() as ctx:
        mt_pool = ctx.enter_context(tc.tile_pool(name="mtp", bufs=1))
        de_pool = ctx.enter_context(tc.tile_pool(name="dep", bufs=1))
        en_pool = ctx.enter_context(tc.tile_pool(name="enp", bufs=3))
        tmp_pool = ctx.enter_context(tc.tile_pool(name="tmpp", bufs=2))
        s_pool = ctx.enter_context(tc.tile_pool(name="sp", bufs=2))
        acc_pool = ctx.enter_context(tc.tile_pool(name="accp", bufs=1))
        ps_pool = ctx.enter_context(tc.tile_pool(name="psp", bufs=3, space="PSUM"))
        ups_pool = ctx.enter_context(tc.tile_pool(name="upsp", bufs=2, space="PSUM"))

        ones = acc_pool.tile([128, 1], f32, tag="ones", name="ones")
        nc.vector.memset(ones[:], 1.0)
        acc = acc_pool.tile([128, NTB * 4], f32, tag="acc", name="acc")

        # Stationary weights: DE tiles [e128, l512] per (b, kb); also reused
        # as the elementwise operand of the numerator path.
        de_sb = {}
        for b in range(B):
            for k in range(NKB):
                t = de_pool.tile([128, L], f32, tag=f"de{b}_{k}", name=f"de{b}_{k}")
                nc.sync.dma_start(out=t[:], in_=de[b, k * 128 : (k + 1) * 128, :])
                de_sb[b, k] = t

        # Moving operand: transposed-M tiles [e128, v500], one per (kb, vb).
        # v-major issue order so the first matmuls' operands arrive first.
        mt_sb = {}
        for v in range(NVB):
            for k in range(NKB):
                t = mt_pool.tile([128, VBLK], f32, tag=f"mt{k}_{v}", name=f"mt{k}_{v}")
                nc.sync.dma_start(
                    out=t[:], in_=mt[k * 128 : (k + 1) * 128, v * VBLK : (v + 1) * VBLK]
                )
                mt_sb[k, v] = t

        # Main loop: per token block, per pair of vocab blocks: accumulate
        # over e into a 2-bank PSUM tile, then one fused exp+sum on ACT.
        for tb in range(NTB):
            b, lb = divmod(tb, NLB)
            for h in range(NVB // 2):
                ps = ps_pool.tile([128, 2, 512], f32, tag="ps", name=f"ps{tb}_{h}")
                for j in range(2):
                    v = h * 2 + j
                    for k in range(NKB):
                        nc.tensor.matmul(
                            ps[:, j, 0:VBLK],
                            lhsT=de_sb[b, k][:, lb * 128 : (lb + 1) * 128],
                            rhs=mt_sb[k, v][:],
                            start=(k == 0),
                            stop=(k == NKB - 1),
                        )
                nc.scalar.activation(
                    out=ps[:, :, 0:VBLK],
                    in_=ps[:, :, 0:VBLK],
                    func=mybir.ActivationFunctionType.Exp,
                    scale=INV_TAU,
                    accum_out=acc[:, tb * 4 + h : tb * 4 + h + 1],
                )

        nc.sync.dma_start(out=down_acc[:, :], in_=acc[:])

        # Numerator path: s[b,l] = sum_e EN*DE via elementwise mul (DVE) and
        # a ones-matmul partition reduction (PE).  Runs in the PE tail.
        for b in range(B):
            ups = ups_pool.tile([1, L], f32, tag="ups", name=f"ups{b}")
            for k in range(NKB):
                et = en_pool.tile([128, L], f32, tag="en", name=f"en{b}_{k}")
                nc.sync.dma_start(out=et[:], in_=en[b, k * 128 : (k + 1) * 128, :])
                tm = tmp_pool.tile([128, L], f32, tag="tmp", name=f"tm{b}_{k}")
                nc.vector.tensor_mul(tm[:], et[:], de_sb[b, k][:])
                nc.tensor.matmul(
                    ups[:],
                    lhsT=ones[:],
                    rhs=tm[:],
                    start=(k == 0),
                    stop=(k == NKB - 1),
                )
            ssb = s_pool.tile([1, L], f32, tag="ssb", name=f"ssb{b}")
            nc.vector.tensor_copy(ssb[:], ups[:])
            nc.sync.dma_start(out=s_out[b : b + 1, :], in_=ssb[:])

    nc.compile()
    return nc


def kernel(embed_EN, embed_DE, embed_M):
    global LAST_RESULTS
    from concourse.bass_utils import run_bass_kernel_spmd

    if "nc" not in _CACHE:
        _CACHE["nc"] = _build()
    nc = _CACHE["nc"]

    en = np.ascontiguousarray(np.asarray(embed_EN, dtype=np.float32))
    de = np.ascontiguousarray(np.asarray(embed_DE, dtype=np.float32))
    mt_full = np.ascontiguousarray(np.asarray(embed_M, dtype=np.float32).T)  # [E, V]

    in_maps = [
        {
            "mt": np.ascontiguousarray(mt_full[:, c * VS : (c + 1) * VS]),
            "de": de,
            "en": en,
        }
        for c in range(NCORES)
    ]

    res = run_bass_kernel_spmd(nc, in_maps, core_ids=list(range(NCORES)))
    LAST_RESULTS = res

    # Gather: all-reduce the partial denominators across cores, then the
    # final division + scalar sum (done in f64 for a clean f32 result).
    acc_sum = np.zeros((128, NTB * 4), np.float64)
    for r in res.results:
        acc_sum += r["down_acc"].astype(np.float64)
    down = acc_sum.reshape(128, NTB, 4).sum(-1)          # [p, tb]
    down = down.T.reshape(B, NLB, 128).reshape(B, L)     # [b, l=lb*128+p]
    s = res.results[0]["s_out"].astype(np.float64)       # [b, l]
    up = np.exp(INV_TAU * s)
    return np.asarray((up / down).sum(), dtype=np.float32)


# revision 2
# speedup vs baseline: 2.8482x; 2.8482x over previous
"""Trainium2 Bass kernel for nn_DeconvCNNLoss.

Computes  sum_{b,l} exp(s[b,l]/tau) / sum_v exp(dist[b,l,v]/tau)
with  dist = einsum('bel,ve->blv', embed_DE, embed_M)
and   s    = sum_e embed_EN * embed_DE.

Sharding: tensor-parallel over the vocab dim V across 8 cores.  Each core
receives embed_M's shard pre-transposed to [E, V/8] (layout choice made on
the host while sharding), the full embed_DE / embed_EN, and produces
partial exp-sum denominators for all B*L tokens plus the numerator dot
products.  The host sums the 8 partial denominators (the "all-reduce"),
applies exp to the numerator and does the final division + scalar sum.

Matmul operands are fed as bf16: trn2 f32 matmuls decompose into LOW/HIGH
PE passes (~4x the cycles measured); bf16 with f32 PSUM accumulation keeps
the final loss within ~1e-3 relative while running the PE at full rate.

Device work per core:
  - 512 bf16 matmuls [128e,128l]^T @ [128e,500v] accumulated over e in PSUM
  - fused exp+row-sum on the scalar engine (activation Exp with accum_out)
  - numerator: DVE elementwise EN*DE, partition-reduced with a ones-matmul
"""

import numpy as np

B, E, L, V = 4, 512, 512, 32000
NCORES = 8
VS = V // NCORES          # 4000 vocab rows per core
VBLK = 500                # vocab columns per matmul (one PSUM bank)
NVB = VS // VBLK          # 8 vocab blocks per core
NLB = L // 128            # 4 token blocks per batch entry
NTB = B * NLB             # 16 token blocks total
NKB = E // 128            # 4 contraction blocks
INV_TAU = 0.1

_CACHE = {}
LAST_RESULTS = None       # test.py reads exec_time_ns from here


def _build():
    from contextlib import ExitStack

    import concourse.bacc as bacc
    import concourse.mybir as mybir
    import concourse.tile as tile

    f32 = mybir.dt.float32
    bf16 = mybir.dt.bfloat16
    nc = bacc.Bacc("TRN2", debug=False, num_devices=NCORES)

    mt = nc.dram_tensor("mt", [E, VS], bf16, kind="ExternalInput").ap()
    de = nc.dram_tensor("de", [B, E, L], bf16, kind="ExternalInput").ap()
    en = nc.dram_tensor("en", [B, E, L], bf16, kind="ExternalInput").ap()
    # down_acc[p, tb*4+h] = sum over one 1000-col vocab slice of exp(dist/tau)
    # for token (b=tb//4, l=(tb%4)*128+p)
    down_acc = nc.dram_tensor("down_acc", [128, NTB * 4], f32, kind="ExternalOutput").ap()
    # s_out[b, l] = sum_e EN[b,e,l]*DE[b,e,l]  (pre-exp numerator dots)
    s_out = nc.dram_tensor("s_out", [B, L], f32, kind="ExternalOutput").ap()

    with tile.TileContext(nc) as tc, ExitStack() as ctx:
        mt_pool = ctx.enter_context(tc.tile_pool(name="mtp", bufs=1))
        de_pool = ctx.enter_context(tc.tile_pool(name="dep", bufs=1))
        en_pool = ctx.enter_context(tc.tile_pool(name="enp", bufs=3))
        tmp_pool = ctx.enter_context(tc.tile_pool(name="tmpp", bufs=2))
        s_pool = ctx.enter_context(tc.tile_pool(name="sp", bufs=2))
        acc_pool = ctx.enter_context(tc.tile_pool(name="accp", bufs=1))
        ps_pool = ctx.enter_context(tc.tile_pool(name="psp", bufs=3, space="PSUM"))
        ups_pool = ctx.enter_context(tc.tile_pool(name="upsp", bufs=2, space="PSUM"))

        ones = acc_pool.tile([128, 1], bf16, tag="ones", name="ones")
        nc.vector.memset(ones[:], 1.0)
        acc = acc_pool.tile([128, NTB * 4], f32, tag="acc", name="acc")

        # Stationary weights: DE tiles [e128, l512] per (b, kb); also reused
        # as the elementwise operand of the numerator path.
        de_sb = {}
        for b in range(B):
            for k in range(NKB):
                t = de_pool.tile([128, L], bf16, tag=f"de{b}_{k}", name=f"de{b}_{k}")
                nc.sync.dma_start(out=t[:], in_=de[b, k * 128 : (k + 1) * 128, :])
                de_sb[b, k] = t

        # Moving operand: transposed-M tiles [e128, v500], one per (kb, vb).
        # v-major issue order so the first matmuls' operands arrive first.
        mt_sb = {}
        for v in range(NVB):
            for k in range(NKB):
                t = mt_pool.tile([128, VBLK], bf16, tag=f"mt{k}_{v}", name=f"mt{k}_{v}")
                nc.sync.dma_start(
                    out=t[:], in_=mt[k * 128 : (k + 1) * 128, v * VBLK : (v + 1) * VBLK]
                )
                mt_sb[k, v] = t

        # Main loop: per token block, per pair of vocab blocks: accumulate
        # over e into a 2-bank PSUM tile, then one fused exp+sum on ACT.
        for tb in range(NTB):
            b, lb = divmod(tb, NLB)
            for h in range(NVB // 2):
                ps = ps_pool.tile([128, 2, 512], f32, tag="ps", name=f"ps{tb}_{h}")
                for j in range(2):
                    v = h * 2 + j
                    for k in range(NKB):
                        nc.tensor.matmul(
                            ps[:, j, 0:VBLK],
                            lhsT=de_sb[b, k][:, lb * 128 : (lb + 1) * 128],
                            rhs=mt_sb[k, v][:],
                            start=(k == 0),
                            stop=(k == NKB - 1),
                        )
                nc.scalar.activation(
                    out=ps[:, :, 0:VBLK],
                    in_=ps[:, :, 0:VBLK],
                    func=mybir.ActivationFunctionType.Exp,
                    scale=INV_TAU,
                    accum_out=acc[:, tb * 4 + h : tb * 4 + h + 1],
                )

        nc.sync.dma_start(out=down_acc[:, :], in_=acc[:])

        # Numerator path: s[b,l] = sum_e EN*DE via elementwise mul (DVE) and
        # a ones-matmul partition reduction (PE).  Runs in the PE tail.
        for b in range(B):
            ups = ups_pool.tile([1, L], f32, tag="ups", name=f"ups{b}")
            for k in range(NKB):
                et = en_pool.tile([128, L], bf16, tag="en", name=f"en{b}_{k}")
                nc.sync.dma_start(out=et[:], in_=en[b, k * 128 : (k + 1) * 128, :])
                tm = tmp_pool.tile([128, L], bf16, tag="tmp", name=f"tm{b}_{k}")
                nc.vector.tensor_mul(tm[:], et[:], de_sb[b, k][:])
                nc.tensor.matmul(
                    ups[:],
                    lhsT=ones[:],
                    rhs=tm[:],
                    start=(k == 0),
                    stop=(k == NKB - 1),
                )
            ssb = s_pool.tile([1, L], f32, tag="ssb", name=f"ssb{b}")
            nc.vector.tensor_copy(ssb[:], ups[:])
            nc.sync.dma_start(out=s_out[b : b + 1, :], in_=ssb[:])

    nc.compile()
    return nc


def kernel(embed_EN, embed_DE, embed_M):
    global LAST_RESULTS
    import ml_dtypes

    from concourse.bass_utils import run_bass_kernel_spmd

    if "nc" not in _CACHE:
        _CACHE["nc"] = _build()
    nc = _CACHE["nc"]

    bf16 = ml_dtypes.bfloat16
    en = np.ascontiguousarray(np.asarray(embed_EN, dtype=np.float32).astype(bf16))
    de = np.ascontiguousarray(np.asarray(embed_DE, dtype=np.float32).astype(bf16))
    mt_full = np.ascontiguousarray(
        np.asarray(embed_M, dtype=np.float32).T.astype(bf16)
    )  # [E, V]

    in_maps = [
        {
            "mt": np.ascontiguousarray(mt_full[:, c * VS : (c + 1) * VS]),
            "de": de,
            "en": en,
        }
        for c in range(NCORES)
    ]

    res = run_bass_kernel_spmd(nc, in_maps, core_ids=list(range(NCORES)))
    LAST_RESULTS = res

    # Gather: all-reduce the partial denominators across cores, then the
    # final division + scalar sum (done in f64 for a clean f32 result).
    acc_sum = np.zeros((128, NTB * 4), np.float64)
    for r in res.results:
        acc_sum += r["down_acc"].astype(np.float64)
    down = acc_sum.reshape(128, NTB, 4).sum(-1)          # [p, tb]
    down = down.T.reshape(B, NLB, 128).reshape(B, L)     # [b, l=lb*128+p]
    s = res.results[0]["s_out"].astype(np.float64)       # [b, l]
    up = np.exp(INV_TAU * s)
    return np.asarray((up / down).sum(), dtype=np.float32)


# revision 6
# speedup vs baseline: 2.9538x; 1.0371x over previous
"""Trainium2 Bass kernel for nn_DeconvCNNLoss.

Computes  sum_{b,l} exp(s[b,l]/tau) / sum_v exp(dist[b,l,v]/tau)
with  dist = einsum('bel,ve->blv', embed_DE, embed_M)
and   s    = sum_e embed_EN * embed_DE.

Sharding: tensor-parallel over the vocab dim V across 8 cores.  Each core
receives embed_M's shard pre-transposed to [E, V/8] (layout choice made on
the host while sharding), the full embed_DE / embed_EN, and produces
partial exp-sum denominators for all B*L tokens plus the numerator dot
products.  The host sums the 8 partial denominators (the "all-reduce"),
applies exp to the numerator and does the final division + scalar sum.

Matmul operands are fed as bf16: trn2 f32 matmuls decompose into LOW/HIGH
PE passes (~4x the cycles measured); bf16 with f32 PSUM accumulation keeps
the final loss within ~1e-3 relative while running the PE at full rate.

Device work per core:
  - 512 bf16 matmuls [128e,128l]^T @ [128e,500v] accumulated over e in PSUM
  - fused exp+row-sum on the scalar engine (activation Exp with accum_out)
  - numerator: DVE elementwise EN*DE, partition-reduced with a ones-matmul
"""

import numpy as np

B, E, L, V = 4, 512, 512, 32000
NCORES = 8
VS = V // NCORES          # 4000 vocab rows per core
VBLK = 500                # vocab columns per matmul (one PSUM bank)
NVB = VS // VBLK          # 8 vocab blocks per core
NLB = L // 128            # 4 token blocks per batch entry
NTB = B * NLB             # 16 token blocks total
NKB = E // 128            # 4 contraction blocks
INV_TAU = 0.1

_CACHE = {}
LAST_RESULTS = None       # test.py reads exec_time_ns from here


def _build():
    from contextlib import ExitStack

    import concourse.bacc as bacc
    import concourse.mybir as mybir
    import concourse.tile as tile

    f32 = mybir.dt.float32
    bf16 = mybir.dt.bfloat16
    nc = bacc.Bacc("TRN2", debug=False, num_devices=NCORES)

    mt = nc.dram_tensor("mt", [E, VS], bf16, kind="ExternalInput").ap()
    de = nc.dram_tensor("de", [B, E, L], bf16, kind="ExternalInput").ap()
    # f32 copies for the numerator path: the final loss is dominated by the
    # largest few exp(s/tau) tokens, so s must be computed at f32 precision.
    def_ = nc.dram_tensor("def", [B, E, L], f32, kind="ExternalInput").ap()
    enf = nc.dram_tensor("enf", [B, E, L], f32, kind="ExternalInput").ap()
    # down_acc[p, tb*4+h] = sum over one 1000-col vocab slice of exp(dist/tau)
    # for token (b=tb//4, l=(tb%4)*128+p)
    down_acc = nc.dram_tensor("down_acc", [128, NTB * 4], f32, kind="ExternalOutput").ap()
    # s_out[b, l] = sum_e EN[b,e,l]*DE[b,e,l]  (pre-exp numerator dots)
    s_out = nc.dram_tensor("s_out", [B, L], f32, kind="ExternalOutput").ap()

    with tile.TileContext(nc) as tc, ExitStack() as ctx:
        mt_pool = ctx.enter_context(tc.tile_pool(name="mtp", bufs=1))
        de_pool = ctx.enter_context(tc.tile_pool(name="dep", bufs=1))
        en_pool = ctx.enter_context(tc.tile_pool(name="enp", bufs=3))
        tmp_pool = ctx.enter_context(tc.tile_pool(name="tmpp", bufs=2))
        s_pool = ctx.enter_context(tc.tile_pool(name="sp", bufs=2))
        acc_pool = ctx.enter_context(tc.tile_pool(name="accp", bufs=1))
        ps_pool = ctx.enter_context(tc.tile_pool(name="psp", bufs=3, space="PSUM"))
        ups_pool = ctx.enter_context(tc.tile_pool(name="upsp", bufs=2, space="PSUM"))

        ones = acc_pool.tile([128, 1], f32, tag="ones", name="ones")
        nc.vector.memset(ones[:], 1.0)
        acc = acc_pool.tile([128, NTB * 4], f32, tag="acc", name="acc")

        # Stationary weights: DE tiles [e128, l512] per (b, kb); also reused
        # as the elementwise operand of the numerator path.
        de_sb = {}
        for b in range(B):
            for k in range(NKB):
                t = de_pool.tile([128, L], bf16, tag=f"de{b}_{k}", name=f"de{b}_{k}")
                nc.sync.dma_start(out=t[:], in_=de[b, k * 128 : (k + 1) * 128, :])
                de_sb[b, k] = t

        # Moving operand: transposed-M tiles [e128, v500], one per (kb, vb).
        # v-major issue order so the first matmuls' operands arrive first.
        mt_sb = {}
        for v in range(NVB):
            for k in range(NKB):
                t = mt_pool.tile([128, VBLK], bf16, tag=f"mt{k}_{v}", name=f"mt{k}_{v}")
                nc.sync.dma_start(
                    out=t[:], in_=mt[k * 128 : (k + 1) * 128, v * VBLK : (v + 1) * VBLK]
                )
                mt_sb[k, v] = t

        # Main loop: per token block, per pair of vocab blocks: accumulate
        # over e into a 2-bank PSUM tile, then one fused exp+sum on ACT.
        for tb in range(NTB):
            b, lb = divmod(tb, NLB)
            for h in range(NVB // 2):
                ps = ps_pool.tile([128, 2, 512], f32, tag="ps", name=f"ps{tb}_{h}")
                for j in range(2):
                    v = h * 2 + j
                    for k in range(NKB):
                        nc.tensor.matmul(
                            ps[:, j, 0:VBLK],
                            lhsT=de_sb[b, k][:, lb * 128 : (lb + 1) * 128],
                            rhs=mt_sb[k, v][:],
                            start=(k == 0),
                            stop=(k == NKB - 1),
                        )
                nc.scalar.activation(
                    out=ps[:, :, 0:VBLK],
                    in_=ps[:, :, 0:VBLK],
                    func=mybir.ActivationFunctionType.Exp,
                    scale=INV_TAU,
                    accum_out=acc[:, tb * 4 + h : tb * 4 + h + 1],
                )

        nc.sync.dma_start(out=down_acc[:, :], in_=acc[:])

        # Numerator path (all f32): tm_k = EN*DE per e-block on DVE, partial
        # partition-group sums folded elementwise, then one ones-matmul per
        # batch entry reduces the remaining 128 partitions.  ~5us PE tail.
        for b in range(B):
            tsum = tmp_pool.tile([128, L], f32, tag="tsum", name=f"tsum{b}")
            for k in range(NKB):
                et = en_pool.tile([128, L], f32, tag="en", name=f"en{b}_{k}")
                nc.sync.dma_start(out=et[:], in_=enf[b, k * 128 : (k + 1) * 128, :])
                dt = en_pool.tile([128, L], f32, tag="def", name=f"def{b}_{k}")
                nc.sync.dma_start(out=dt[:], in_=def_[b, k * 128 : (k + 1) * 128, :])
                if k == 0:
                    nc.vector.tensor_mul(tsum[:], et[:], dt[:])
                else:
                    tm = tmp_pool.tile([128, L], f32, tag="tmp", name=f"tm{b}_{k}")
                    nc.vector.tensor_mul(tm[:], et[:], dt[:])
                    nc.vector.tensor_add(tsum[:], tsum[:], tm[:])
            ups = ups_pool.tile([1, L], f32, tag="ups", name=f"ups{b}")
            nc.tensor.matmul(ups[:], lhsT=ones[:], rhs=tsum[:], start=True, stop=True)
            ssb = s_pool.tile([1, L], f32, tag="ssb", name=f"ssb{b}")
            nc.vector.tensor_copy(ssb[:], ups[:])
            nc.sync.dma_start(out=s_out[b : b + 1, :], in_=ssb[:])

    nc.compile()
    return nc


def kernel(embed_EN, embed_DE, embed_M):
    global LAST_RESULTS
    import ml_dtypes

    from concourse.bass_utils import run_bass_kernel_spmd

    if "nc" not in _CACHE:
        _CACHE["nc"] = _build()
    nc = _CACHE["nc"]

    bf16 = ml_dtypes.bfloat16
    enf = np.ascontiguousarray(np.asarray(embed_EN, dtype=np.float32))
    def_ = np.ascontiguousarray(np.asarray(embed_DE, dtype=np.float32))
    de = np.ascontiguousarray(def_.astype(bf16))
    mt_full = np.ascontiguousarray(
        np.asarray(embed_M, dtype=np.float32).T.astype(bf16)
    )  # [E, V]

    in_maps = [
        {
            "mt": np.ascontiguousarray(mt_full[:, c * VS : (c + 1) * VS]),
            "de": de,
            "def": def_,
            "enf": enf,
        }
        for c in range(NCORES)
    ]

    res = run_bass_kernel_spmd(nc, in_maps, core_ids=list(range(NCORES)))
    LAST_RESULTS = res

    # Gather: all-reduce the partial denominators across cores, then the
    # final division + scalar sum (done in f64 for a clean f32 result).
    acc_sum = np.zeros((128, NTB * 4), np.float64)
    for r in res.results:
        acc_sum += r["down_acc"].astype(np.float64)
    down = acc_sum.reshape(128, NTB, 4).sum(-1)          # [p, tb]
    down = down.T.reshape(B, NLB, 128).reshape(B, L)     # [b, l=lb*128+p]
    s = res.results[0]["s_out"].astype(np.float64)       # [b, l]
    up = np.exp(INV_TAU * s)
    return np.asarray((up / down).sum(), dtype=np.float32)


# revision 10
# speedup vs baseline: 3.5556x; 1.2037x over previous
"""Trainium2 Bass kernel for nn_DeconvCNNLoss.

Computes  sum_{b,l} exp(s[b,l]/tau) / sum_v exp(dist[b,l,v]/tau)
with  dist = einsum('bel,ve->blv', embed_DE, embed_M)
and   s    = sum_e embed_EN * embed_DE.

Sharding: tensor-parallel over the vocab dim V across 8 cores.  Each core
receives embed_M's shard pre-transposed to [E, V/8] (layout choice made on
the host while sharding), the full embed_DE / embed_EN, and produces
partial exp-sum denominators for all B*L tokens plus the numerator dot
products.  The host sums the 8 partial denominators (the "all-reduce"),
applies exp to the numerator and does the final division + scalar sum.

Matmul operands are fed as bf16: trn2 f32 matmuls decompose into LOW/HIGH
PE passes (~4x the cycles measured); bf16 with f32 PSUM accumulation keeps
the final loss within ~1e-3 relative while running the PE at full rate.

Device work per core:
  - 512 bf16 matmuls [128e,128l]^T @ [128e,500v] accumulated over e in PSUM
  - fused exp+row-sum on the scalar engine (activation Exp with accum_out)
  - numerator: DVE elementwise EN*DE, partition-reduced with a ones-matmul
"""

import numpy as np

B, E, L, V = 4, 512, 512, 32000
NCORES = 8
VS = V // NCORES          # 4000 vocab rows per core
VBLK = 500                # vocab columns per matmul (one PSUM bank)
NVB = VS // VBLK          # 8 vocab blocks per core
NLB = L // 128            # 4 token blocks per batch entry
NTB = B * NLB             # 16 token blocks total
NKB = E // 128            # 4 contraction blocks
INV_TAU = 0.1

_CACHE = {}
LAST_RESULTS = None       # test.py reads exec_time_ns from here


def _build():
    from contextlib import ExitStack

    import concourse.bacc as bacc
    import concourse.mybir as mybir
    import concourse.tile as tile

    f32 = mybir.dt.float32
    bf16 = mybir.dt.bfloat16
    nc = bacc.Bacc("TRN2", debug=False, num_devices=NCORES)

    mt = nc.dram_tensor("mt", [E, VS], bf16, kind="ExternalInput").ap()
    de = nc.dram_tensor("de", [B, E, L], bf16, kind="ExternalInput").ap()
    # f32 copies for the numerator path: the final loss is dominated by the
    # largest few exp(s/tau) tokens, so s must be computed at f32 precision.
    def_ = nc.dram_tensor("def", [B, E, L], f32, kind="ExternalInput").ap()
    enf = nc.dram_tensor("enf", [B, E, L], f32, kind="ExternalInput").ap()
    # down_acc[p, tb*4+h] = sum over one 1000-col vocab slice of exp(dist/tau)
    # for token (b=tb//4, l=(tb%4)*128+p)
    down_acc = nc.dram_tensor("down_acc", [128, NTB * 4], f32, kind="ExternalOutput").ap()
    # s_out[b, l] = sum_e EN[b,e,l]*DE[b,e,l]  (pre-exp numerator dots)
    s_out = nc.dram_tensor("s_out", [B, L], f32, kind="ExternalOutput").ap()

    with tile.TileContext(nc) as tc, ExitStack() as ctx:
        mt_pool = ctx.enter_context(tc.tile_pool(name="mtp", bufs=1))
        de_pool = ctx.enter_context(tc.tile_pool(name="dep", bufs=1))
        en_pool = ctx.enter_context(tc.tile_pool(name="enp", bufs=2))
        tmp_pool = ctx.enter_context(tc.tile_pool(name="tmpp", bufs=2))
        s_pool = ctx.enter_context(tc.tile_pool(name="sp", bufs=2))
        acc_pool = ctx.enter_context(tc.tile_pool(name="accp", bufs=1))
        ps_pool = ctx.enter_context(tc.tile_pool(name="psp", bufs=3, space="PSUM"))
        ups_pool = ctx.enter_context(tc.tile_pool(name="upsp", bufs=2, space="PSUM"))

        ones = acc_pool.tile([128, 1], f32, tag="ones", name="ones")
        nc.vector.memset(ones[:], 1.0)
        acc = acc_pool.tile([128, NTB * 4], f32, tag="acc", name="acc")

        # DRAM views folding the e dim as (k p): partition p, e-block k.
        de_r = de.rearrange("b (k p) l -> b p k l", p=128)
        mt_r = mt.rearrange("(k p) v -> p k v", p=128)
        enf_r = enf.rearrange("b (k p) l -> b p k l", p=128)
        def_r = def_.rearrange("b (k p) l -> b p k l", p=128)

        # Stationary weights: one [p128, k4, l512] tile per batch entry,
        # loaded in a single DMA.  b=0 first: the first matmuls need it.
        de_sb = {}
        for b in range(B):
            t = de_pool.tile([128, NKB, L], bf16, tag=f"de{b}", name=f"de{b}")
            nc.sync.dma_start(out=t[:], in_=de_r[b])
            de_sb[b] = t
            if b == 0:
                # Moving operand: transposed-M tiles [p128, k4, v500], one
                # DMA per vocab block, issued right after de0 so matmuls
                # can start as soon as the first block lands.
                mt_sb = {}
                for v in range(NVB):
                    t2 = mt_pool.tile([128, NKB, VBLK], bf16, tag=f"mt{v}", name=f"mt{v}")
                    nc.sync.dma_start(
                        out=t2[:], in_=mt_r[:, :, v * VBLK : (v + 1) * VBLK]
                    )
                    mt_sb[v] = t2

        # Main loop: per token block, per pair of vocab blocks: accumulate
        # over e into a 2-bank PSUM tile, then one fused exp+sum on ACT.
        for tb in range(NTB):
            b, lb = divmod(tb, NLB)
            for h in range(NVB // 2):
                ps = ps_pool.tile([128, 2, 512], f32, tag="ps", name=f"ps{tb}_{h}")
                for j in range(2):
                    v = h * 2 + j
                    for k in range(NKB):
                        nc.tensor.matmul(
                            ps[:, j, 0:VBLK],
                            lhsT=de_sb[b][:, k, lb * 128 : (lb + 1) * 128],
                            rhs=mt_sb[v][:, k, :],
                            start=(k == 0),
                            stop=(k == NKB - 1),
                        )
                nc.scalar.activation(
                    out=ps[:, :, 0:VBLK],
                    in_=ps[:, :, 0:VBLK],
                    func=mybir.ActivationFunctionType.Exp,
                    scale=INV_TAU,
                    accum_out=acc[:, tb * 4 + h : tb * 4 + h + 1],
                )

        nc.sync.dma_start(out=down_acc[:, :], in_=acc[:])

        # Numerator path (all f32): tm_k = EN*DE per e-block on DVE, partial
        # partition-group sums folded elementwise into per-b tsum tiles (all
        # DVE work runs during the main loop), then one ones-matmul per
        # batch entry at the PE tail reduces the remaining 128 partitions.
        tsum_sb = {}
        for b in range(B):
            et = en_pool.tile([128, NKB, L], f32, tag="en", name=f"en{b}")
            nc.sync.dma_start(out=et[:], in_=enf_r[b])
            dt = en_pool.tile([128, NKB, L], f32, tag="def", name=f"def{b}")
            nc.sync.dma_start(out=dt[:], in_=def_r[b])
            tsum = tmp_pool.tile([128, L], f32, tag=f"tsum{b}", name=f"tsum{b}")
            tsum_sb[b] = tsum
            for k in range(NKB):
                if k == 0:
                    nc.vector.tensor_mul(tsum[:], et[:, 0, :], dt[:, 0, :])
                else:
                    tm = tmp_pool.tile([128, L], f32, tag="tmp", name=f"tm{b}_{k}")
                    nc.vector.tensor_mul(tm[:], et[:, k, :], dt[:, k, :])
                    nc.vector.tensor_add(tsum[:], tsum[:], tm[:])
        for b in range(B):
            ups = ups_pool.tile([1, L], f32, tag="ups", name=f"ups{b}")
            nc.tensor.matmul(
                ups[:], lhsT=ones[:], rhs=tsum_sb[b][:], start=True, stop=True
            )
            ssb = s_pool.tile([1, L], f32, tag="ssb", name=f"ssb{b}")
            nc.vector.tensor_copy(ssb[:], ups[:])
            nc.sync.dma_start(out=s_out[b : b + 1, :], in_=ssb[:])

    nc.compile()
    return nc


def kernel(embed_EN, embed_DE, embed_M):
    global LAST_RESULTS
    import ml_dtypes

    from concourse.bass_utils import run_bass_kernel_spmd

    if "nc" not in _CACHE:
        _CACHE["nc"] = _build()
    nc = _CACHE["nc"]

    bf16 = ml_dtypes.bfloat16
    enf = np.ascontiguousarray(np.asarray(embed_EN, dtype=np.float32))
    def_ = np.ascontiguousarray(np.asarray(embed_DE, dtype=np.float32))
    de = np.ascontiguousarray(def_.astype(bf16))
    mt_full = np.ascontiguousarray(
        np.asarray(embed_M, dtype=np.float32).T.astype(bf16)
    )  # [E, V]

    in_maps = [
        {
            "mt": np.ascontiguousarray(mt_full[:, c * VS : (c + 1) * VS]),
            "de": de,
            "def": def_,
            "enf": enf,
        }
        for c in range(NCORES)
    ]

    res = run_bass_kernel_spmd(nc, in_maps, core_ids=list(range(NCORES)))
    LAST_RESULTS = res

    # Gather: all-reduce the partial denominators across cores, then the
    # final division + scalar sum (done in f64 for a clean f32 result).
    acc_sum = np.zeros((128, NTB * 4), np.float64)
    for r in res.results:
        acc_sum += r["down_acc"].astype(np.float64)
    down = acc_sum.reshape(128, NTB, 4).sum(-1)          # [p, tb]
    down = down.T.reshape(B, NLB, 128).reshape(B, L)     # [b, l=lb*128+p]
    s = res.results[0]["s_out"].astype(np.float64)       # [b, l]
    up = np.exp(INV_TAU * s)
    return np.asarray((up / down).sum(), dtype=np.float32)
